# revision 6
# baseline (speedup 1.0000x reference)
"""Softmax-weighted nearest-neighbor aggregation (DiffusionStar) on 8 TRN2 cores.

Strategy (v2):
  - Shard the train set (N=50000) across 8 cores (6250 rows each, padded to 6272).
  - Two-phase softmax per core, but with three DMA-traffic optimizations over
    the v1 baseline (which streamed the 38.4MB train slice twice = 76.8MB):
      1. The transposed train copy (tT) is stored group-contiguous in DRAM
         ([128 partitions x 24KB contiguous] per 512-col group) so each load is
         128 large descriptors instead of 3072 x 1KB ones (descriptor-limited
         at ~288GB/s before; ~full rate after).
      2. -||t||^2 is folded into the score GEMM as a K=1 fp32 matmul
         (lhsT = -ones[1,64], rhs = trsq[1,512]) accumulating into the same
         PSUM group, killing the [64, N_PAD] fp32 broadcast input and the DVE
         subtract.
      3. 14 of the 49 phase-2 natural-layout chunks are NOT re-streamed from
         DRAM: their kT tiles (already in SBUF for phase 1) are transposed
         on-chip into cached natural tiles — half via PE identity-transpose
         (PSUM staging + DVE/ACT copies), half via DVE 32x32 StreamTranspose
         instructions that do the full 128x128 transpose in 16 cross-
         partition-group block ops (SBUF->SBUF, no PSUM). This happens in
         phase 1's DMA-bound shadow, so phase 2 only streams 35 chunks.
  - Host merges (M, S, ACC) across cores with the online-softmax combine and
    applies the final coefficients in fp64.

Numerics: identical math to v1 (validated 5e-4): fp16 GEMMs, fp32 PSUM, exact
fp32 trsq subtraction (now inside the PE accumulation), fp16 p with fp32 sums.
"""

import numpy as np

B = 64
D = 3072
N = 50000
NCORES = 8
N_LOC = N // NCORES          # 6250
N_PAD = 6272                 # 49 * 128
KD = D // 128                # 24
KN = N_PAD // 128            # 49
DJ = D // 512                # 6
GROUPS = [(i * 512, 512) for i in range(12)] + [(6144, 128)]
NG = len(GROUPS)
PAD_TRSQ = 1e9
NAT_BUFS = 5

# chunks whose natural-layout tile is produced on-chip instead of re-streamed
CACHED = [c for c in range(KN) if c % 4 == 1]            # 12 chunks, 1 per group
CACHED_PE = set(CACHED[0::2])                            # PE identity-transpose
CACHED_DVE = set(CACHED[1::2])                           # DVE stream-transpose

_CACHED = {}


def _build_nc():
    import concourse.bacc as bacc
    import concourse.tile as tile
    from concourse import mybir
    from contextlib import ExitStack

    f16 = mybir.dt.float16
    f32 = mybir.dt.float32

    nc = bacc.Bacc("TRN2", target_bir_lowering=False, debug=False)

    # group-contiguous transposed train: groups 0..11 (512 wide)
    tTg = nc.dram_tensor("tTg", [12 * 128, KD * 512], f16, kind="ExternalInput").ap()
    # group 12 (128 wide), compact
    tTl = nc.dram_tensor("tTl", [128, KD * 128], f16, kind="ExternalInput").ap()
    tn = nc.dram_tensor("tn", [N_PAD, D], f16, kind="ExternalInput").ap()
    xT = nc.dram_tensor("xT", [D, B], f16, kind="ExternalInput").ap()
    ident = nc.dram_tensor("ident", [128, 128], f16, kind="ExternalInput").ap()
    trsq = nc.dram_tensor("trsq", [96, 5 * 512], f32, kind="ExternalInput").ap()
    negs = nc.dram_tensor("negs", [96, B], f32, kind="ExternalInput").ap()
    gcol = nc.dram_tensor("gcol", [B, 1], f32, kind="ExternalInput").ap()

    acc_out = nc.dram_tensor("acc_out", [B, D], f32, kind="ExternalOutput").ap()
    s_out = nc.dram_tensor("s_out", [B, 1], f32, kind="ExternalOutput").ap()
    m_out = nc.dram_tensor("m_out", [B, 1], f32, kind="ExternalOutput").ap()

    with tile.TileContext(nc) as tc, ExitStack() as ctx:
        const = ctx.enter_context(tc.tile_pool(name="const", bufs=1))
        kTp = ctx.enter_context(tc.tile_pool(name="kT", bufs=2))
        cachep = ctx.enter_context(tc.tile_pool(name="cache", bufs=1))
        natp = ctx.enter_context(tc.tile_pool(name="nat", bufs=NAT_BUFS))
        sb = ctx.enter_context(tc.tile_pool(name="sb", bufs=1))
        pTp = ctx.enter_context(tc.tile_pool(name="pTp", bufs=4))
        pp = ctx.enter_context(tc.tile_pool(name="pp", bufs=2))

        # --- constants ---
        xT_sb = const.tile([128, KD, B], f16)
        nc.sync.dma_start(xT_sb[:], xT.rearrange("(k p) b -> p k b", p=128))
        id_sb = const.tile([128, 128], f16)
        nc.sync.dma_start(id_sb[:], ident[:])
        trsq_sb = const.tile([96, 5, 512], f32)
        nc.sync.dma_start(trsq_sb[:], trsq.rearrange("p (s n) -> p s n", n=512))
        neg_sb = const.tile([96, B], f32)
        nc.sync.dma_start(neg_sb[:], negs[:])
        g_sb = const.tile([B, 1], f32)
        nc.sync.dma_start(g_sb[:], gcol[:])

        mpart = sb.tile([B, NG], f32)
        ssum = sb.tile([B, NG], f32)
        stat = sb.tile([B, 4], f32)
        acc_sb = sb.tile([B, D], f32)
        sc_tiles = []
        p_tiles = []
        nat_cache = {}

        # ---------------- phase 1: scores + per-group max + cached transposes
        with tc.tile_pool(name="psS", bufs=2, space="PSUM") as psS, \
             tc.tile_pool(name="psT", bufs=2, space="PSUM") as psT:
            for gi, (n0, W) in enumerate(GROUPS):
                kT = kTp.tile([128, KD, 512], f16, tag="kT")
                if gi < 12:
                    nc.sync.dma_start(
                        kT[:, :, :W],
                        tTg[gi * 128:(gi + 1) * 128, :].rearrange(
                            "p (k n) -> p k n", n=512))
                else:
                    nc.sync.dma_start(
                        kT[:, :, :W],
                        tTl[:, :].rearrange("p (k n) -> p k n", n=128))

                ps = psS.tile([B, 512], f32, tag="ps")
                # -||t||^2 via K=1 fp32 matmul (negs.T @ trsq broadcast);
                # group gi's trsq row lives at partition 32*(gi%3), slot gi//3
                # so the base partition is a legal 0/32/64.
                bp = 32 * (gi % 3)
                nc.tensor.matmul(ps[:, :W], neg_sb[bp:bp + 1, :],
                                 trsq_sb[bp:bp + 1, gi // 3, :W],
                                 start=True, stop=False)
                for k in range(KD):
                    nc.tensor.matmul(ps[:, :W], xT_sb[:, k, :], kT[:, k, :W],
                                     start=False, stop=(k == KD - 1))
                sc = sb.tile([B, 512], f32, tag=f"sc{gi}")
                sc_tiles.append(sc)
                nc.vector.tensor_copy(sc[:, :W], ps[:, :W])
                nc.vector.reduce_max(mpart[:, gi:gi + 1], sc[:, :W],
                                     axis=mybir.AxisListType.X)

                # on-chip transposes of cached chunks of this group
                for ci in range(W // 128):
                    c = 4 * gi + ci
                    if c not in CACHED:
                        continue
                    nat = cachep.tile([128, D], f16, tag=f"natc{c}")
                    nat_cache[c] = nat
                    if c in CACHED_PE:
                        for k in range(KD):
                            pt = psT.tile([128, 128], f16, tag="pt")
                            nc.tensor.transpose(
                                pt[:], kT[:, k, ci * 128:(ci + 1) * 128],
                                id_sb[:])
                            if k % 2 == 0:
                                nc.vector.tensor_copy(
                                    nat[:, k * 128:(k + 1) * 128], pt[:])
                            else:
                                nc.scalar.copy(
                                    nat[:, k * 128:(k + 1) * 128], pt[:])
                    else:
                        natv = nat.rearrange("p (k d) -> p k d", d=128)
                        for i in range(4):
                            for j in range(4):
                                nc.vector.transpose(
                                    natv[32 * j:32 * j + 32, :,
                                         32 * i:32 * i + 32],
                                    kT[32 * i:32 * i + 32, :,
                                       ci * 128 + 32 * j:ci * 128 + 32 * j + 32])

        # --- global max, bias = -g*M ---
        nc.vector.reduce_max(stat[:, 0:1], mpart[:, :NG],
                             axis=mybir.AxisListType.X)
        nc.vector.tensor_tensor(stat[:, 2:3], g_sb[:], stat[:, 0:1],
                                op=mybir.AluOpType.mult)
        nc.vector.tensor_scalar_mul(stat[:, 2:3], stat[:, 2:3], -1.0)

        # ---------------- phase 2: exp -> pT -> GEMM2 ----------------
        with tc.tile_pool(name="psT2", bufs=2, space="PSUM") as psT2, \
             tc.tile_pool(name="psA", bufs=1, space="PSUM") as psA:
            acc_ps = psA.tile([B, DJ, 512], f32)
            for c in range(KN):
                gi = c // 4
                ci = c % 4
                n0, W = GROUPS[gi]
                if ci == 0:
                    p = pp.tile([B, 512], f16, tag="p")
                    p_tiles.append(p)
                    nc.scalar.activation(p[:, :W], sc_tiles[gi][:, :W],
                                         mybir.ActivationFunctionType.Exp,
                                         bias=stat[:, 2:3], scale=g_sb[:],
                                         accum_out=ssum[:, gi:gi + 1])
                pt2 = psT2.tile([128, B], f16, tag="pt2")
                nc.tensor.transpose(pt2[:],
                                    p_tiles[gi][:, ci * 128:(ci + 1) * 128],
                                    id_sb[:B, :B])
                pT = pTp.tile([128, B], f16, tag="pT")
                nc.vector.tensor_copy(pT[:], pt2[:])
                if c in nat_cache:
                    nat = nat_cache[c]
                else:
                    nat = natp.tile([128, D], f16, tag="nat")
                    nc.sync.dma_start(nat[:], tn[c * 128:(c + 1) * 128, :])
                for j in range(DJ):
                    nc.tensor.matmul(acc_ps[:, j, :], pT[:],
                                     nat[:, j * 512:(j + 1) * 512],
                                     start=(c == 0), stop=(c == KN - 1))
            for j in range(DJ):
                nc.scalar.copy(acc_sb[:, j * 512:(j + 1) * 512],
                               acc_ps[:, j, :])
                nc.sync.dma_start(acc_out[:, j * 512:(j + 1) * 512],
                                  acc_sb[:, j * 512:(j + 1) * 512])

        nc.vector.reduce_sum(stat[:, 1:2], ssum[:, :NG],
                             axis=mybir.AxisListType.X)
        nc.sync.dma_start(s_out[:], stat[:, 1:2])
        nc.sync.dma_start(m_out[:], stat[:, 0:1])

    nc.compile()
    return nc


def _get_nc():
    if "nc" not in _CACHED:
        _CACHED["nc"] = _build_nc()
    return _CACHED["nc"]


def kernel(x, train, alphas_cumprod, t, **_unused):
    from concourse.bass_utils import run_bass_kernel_spmd

    x = np.asarray(x)
    train = np.asarray(train)
    alphas_cumprod = np.asarray(alphas_cumprod)
    t = np.asarray(t).astype(np.int64)

    xf = x.reshape(B, -1).astype(np.float32)
    tf = train.reshape(N, -1).astype(np.float32)

    acp_t = alphas_cumprod.astype(np.float64)[t]
    a = np.sqrt(acp_t)
    om = 1.0 - acp_t
    gp32 = (a * a / (2.0 * om)).astype(np.float32)   # softmax scale on s''
    xscale = (2.0 / a).astype(np.float32)            # fold into x

    trsq_full = np.einsum("nd,nd->n", tf.astype(np.float64),
                          tf.astype(np.float64)).astype(np.float32)

    t16 = tf.astype(np.float16)
    x16T = np.ascontiguousarray(
        (xscale[:, None] * xf).astype(np.float16).T)  # [D, B]
    ident = np.eye(128, dtype=np.float16)
    g_col = gp32.reshape(B, 1)
    negs = np.zeros((96, B), dtype=np.float32)
    negs[[0, 32, 64], :] = -1.0

    in_maps = []
    for c in range(NCORES):
        sl = slice(c * N_LOC, (c + 1) * N_LOC)
        nat = np.zeros((N_PAD, D), dtype=np.float16)
        nat[:N_LOC] = t16[sl]
        # group-contiguous transposed layout: per group g, [128 d-part, 24 k, W n]
        tTg = np.empty((12, 128, KD, 512), dtype=np.float16)
        for g in range(12):
            blk = nat[g * 512:(g + 1) * 512, :]          # [512 n, 3072 d]
            tTg[g] = blk.reshape(512, KD, 128).transpose(2, 1, 0)
        tTl = nat[6144:6272, :].reshape(128, KD, 128).transpose(2, 1, 0)
        trsq_c = np.full((N_PAD,), PAD_TRSQ, dtype=np.float32)
        trsq_c[:N_LOC] = trsq_full[sl]
        trsqg = np.full((96, 5, 512), PAD_TRSQ, dtype=np.float32)
        for g in range(NG):
            w = GROUPS[g][1]
            trsqg[32 * (g % 3), g // 3, :w] = trsq_c[g * 512:g * 512 + w]
        in_maps.append({
            "tTg": np.ascontiguousarray(tTg.reshape(12 * 128, KD * 512)),
            "tTl": np.ascontiguousarray(tTl.reshape(128, KD * 128)),
            "tn": nat,
            "xT": x16T,
            "ident": ident,
            "trsq": trsqg.reshape(96, 5 * 512),
            "negs": negs,
            "gcol": g_col,
        })

    nc = _get_nc()
    res = run_bass_kernel_spmd(nc, in_maps, list(range(NCORES)))
    _CACHED["last_results"] = res

    # --- host-side online-softmax merge across cores (fp64) ---
    g64 = gp32.astype(np.float64)
    Ms = np.stack([res.results[c]["m_out"][:, 0].astype(np.float64)
                   for c in range(NCORES)])          # [C, B]
    Ss = np.stack([res.results[c]["s_out"][:, 0].astype(np.float64)
                   for c in range(NCORES)])          # [C, B]
    ACCs = np.stack([res.results[c]["acc_out"].astype(np.float64)
                     for c in range(NCORES)])        # [C, B, D]
    Mg = Ms.max(axis=0)                              # [B]
    scale = np.exp(g64[None, :] * (Ms - Mg[None, :]))  # [C, B]
    den = (scale * Ss).sum(axis=0)                   # [B]
    num = (scale[:, :, None] * ACCs).sum(axis=0)     # [C, B, D] -> [B, D]
    weighted = num / den[:, None]

    coef_x = 1.0 / np.sqrt(om)
    coef_x_hat = a / np.sqrt(om)
    out = coef_x[:, None] * xf.astype(np.float64) - coef_x_hat[:, None] * weighted
    return out.reshape(x.shape).astype(np.float32)


# revision 8
# speedup vs baseline: 1.1223x; 1.1223x over previous
"""Softmax-weighted nearest-neighbor aggregation (DiffusionStar) on 8 TRN2 cores.

Strategy (v2):
  - Shard the train set (N=50000) across 8 cores (6250 rows each, padded to 6272).
  - Two-phase softmax per core, but with three DMA-traffic optimizations over
    the v1 baseline (which streamed the 38.4MB train slice twice = 76.8MB):
      1. The transposed train copy (tT) is stored group-contiguous in DRAM
         ([128 partitions x 24KB contiguous] per 512-col group) so each load is
         128 large descriptors instead of 3072 x 1KB ones (descriptor-limited
         at ~288GB/s before; ~full rate after).
      2. -||t||^2 is folded into the score GEMM as a K=1 fp32 matmul
         (lhsT = -ones[1,64], rhs = trsq[1,512]) accumulating into the same
         PSUM group, killing the [64, N_PAD] fp32 broadcast input and the DVE
         subtract.
      3. 14 of the 49 phase-2 natural-layout chunks are NOT re-streamed from
         DRAM: their kT tiles (already in SBUF for phase 1) are transposed
         on-chip into cached natural tiles — half via PE identity-transpose
         (PSUM staging + DVE/ACT copies), half via DVE 32x32 StreamTranspose
         instructions that do the full 128x128 transpose in 16 cross-
         partition-group block ops (SBUF->SBUF, no PSUM). This happens in
         phase 1's DMA-bound shadow, so phase 2 only streams 35 chunks.
  - Host merges (M, S, ACC) across cores with the online-softmax combine and
    applies the final coefficients in fp64.

Numerics: identical math to v1 (validated 5e-4): fp16 GEMMs, fp32 PSUM, exact
fp32 trsq subtraction (now inside the PE accumulation), fp16 p with fp32 sums.
"""

import numpy as np

B = 64
D = 3072
N = 50000
NCORES = 8
N_LOC = N // NCORES          # 6250
N_PAD = 6272                 # 49 * 128
KD = D // 128                # 24
KN = N_PAD // 128            # 49
DJ = D // 512                # 6
GROUPS = [(i * 512, 512) for i in range(12)] + [(6144, 128)]
NG = len(GROUPS)
W_LAST = N_LOC - 6144     # 106 valid columns in the last group (rest is pad)
NAT_BUFS = 3

# chunks whose natural-layout tile is produced on-chip instead of re-streamed
# (one per group for groups 1..10, avoiding pipeline head/tail)
CACHED = [4 * g + 1 for g in range(1, 11)]               # 10 chunks
CACHED_PE = set(CACHED[0::5] + CACHED[2::5] + CACHED[3::5])   # 6 via PE
CACHED_DVE = set(CACHED) - CACHED_PE                          # 4 via DVE

_CACHED = {}


def _build_nc():
    import concourse.bacc as bacc
    import concourse.tile as tile
    from concourse import mybir
    from contextlib import ExitStack

    f16 = mybir.dt.float16
    f32 = mybir.dt.float32

    nc = bacc.Bacc("TRN2", target_bir_lowering=False, debug=False)

    # group-contiguous transposed train: groups 0..11 (512 wide)
    tTg = nc.dram_tensor("tTg", [12 * 128, KD * 512], f16, kind="ExternalInput").ap()
    # group 12 (128 wide), compact
    tTl = nc.dram_tensor("tTl", [128, KD * 128], f16, kind="ExternalInput").ap()
    tn = nc.dram_tensor("tn", [N_PAD, D], f16, kind="ExternalInput").ap()
    xT = nc.dram_tensor("xT", [D, B], f16, kind="ExternalInput").ap()
    ident = nc.dram_tensor("ident", [128, 128], f16, kind="ExternalInput").ap()
    # hi/lo fp16 split of ||t||^2, rows (32m, 32m+1) hold group (3s+m)'s
    # (hi, lo) so the K=2 matmul rhs has a legal base partition
    trsq = nc.dram_tensor("trsq", [96, 5 * 512], f16, kind="ExternalInput").ap()
    negs = nc.dram_tensor("negs", [96, B], f16, kind="ExternalInput").ap()
    gcol = nc.dram_tensor("gcol", [B, 1], f32, kind="ExternalInput").ap()

    acc_out = nc.dram_tensor("acc_out", [B, D], f32, kind="ExternalOutput").ap()
    s_out = nc.dram_tensor("s_out", [B, 1], f32, kind="ExternalOutput").ap()
    m_out = nc.dram_tensor("m_out", [B, 1], f32, kind="ExternalOutput").ap()

    with tile.TileContext(nc) as tc, ExitStack() as ctx:
        const = ctx.enter_context(tc.tile_pool(name="const", bufs=1))
        kTp = ctx.enter_context(tc.tile_pool(name="kT", bufs=3))
        cachep = ctx.enter_context(tc.tile_pool(name="cache", bufs=1))
        natp = ctx.enter_context(tc.tile_pool(name="nat", bufs=NAT_BUFS))
        sb = ctx.enter_context(tc.tile_pool(name="sb", bufs=1))
        pTp = ctx.enter_context(tc.tile_pool(name="pTp", bufs=4))
        pp = ctx.enter_context(tc.tile_pool(name="pp", bufs=2))

        # --- constants ---
        xT_sb = const.tile([128, KD, B], f16)
        nc.sync.dma_start(xT_sb[:], xT.rearrange("(k p) b -> p k b", p=128))
        id_sb = const.tile([128, 128], f16)
        nc.sync.dma_start(id_sb[:], ident[:])
        trsq_sb = const.tile([96, 5, 512], f16)
        nc.sync.dma_start(trsq_sb[:], trsq.rearrange("p (s n) -> p s n", n=512))
        neg_sb = const.tile([96, B], f16)
        nc.sync.dma_start(neg_sb[:], negs[:])
        g_sb = const.tile([B, 1], f32)
        nc.sync.dma_start(g_sb[:], gcol[:])

        mpart = sb.tile([B, NG], f32)
        ssum = sb.tile([B, NG], f32)
        stat = sb.tile([B, 4], f32)
        acc_sb = sb.tile([B, D], f32)
        sc_tiles = []
        p_tiles = []
        nat_cache = {}

        # ---------------- phase 1: scores + per-group max + cached transposes
        def emit_transposes(kT_t, gi_t):
            n0_t, W_t = GROUPS[gi_t]
            for ci in range(W_t // 128):
                c = 4 * gi_t + ci
                if c not in CACHED:
                    continue
                nat = cachep.tile([128, D], f16, tag=f"natc{c}")
                nat_cache[c] = nat
                if c in CACHED_PE:
                    for k in range(KD):
                        pt = psT.tile([128, 128], f16, tag="pt")
                        nc.tensor.transpose(
                            pt[:], kT_t[:, k, ci * 128:(ci + 1) * 128],
                            id_sb[:])
                        nc.scalar.copy(nat[:, k * 128:(k + 1) * 128], pt[:])
                else:
                    natv = nat.rearrange("p (k d) -> p k d", d=128)
                    for i in range(4):
                        for j in range(4):
                            nc.vector.transpose(
                                natv[32 * j:32 * j + 32, :,
                                     32 * i:32 * i + 32],
                                kT_t[32 * i:32 * i + 32, :,
                                     ci * 128 + 32 * j:ci * 128 + 32 * j + 32])

        with tc.tile_pool(name="psS", bufs=2, space="PSUM") as psS, \
             tc.tile_pool(name="psT", bufs=4, space="PSUM") as psT:
            pending = None
            for gi, (n0, W) in enumerate(GROUPS):
                kT = kTp.tile([128, KD, 512], f16, tag="kT")
                if gi < 12:
                    nc.sync.dma_start(
                        kT[:, :, :W],
                        tTg[gi * 128:(gi + 1) * 128, :].rearrange(
                            "p (k n) -> p k n", n=512))
                else:
                    nc.sync.dma_start(
                        kT[:, :, :W],
                        tTl[:, :].rearrange("p (k n) -> p k n", n=128))

                ps = psS.tile([B, 512], f32, tag="ps")
                # -||t||^2 via K=2 fp16 matmul: rhs rows = (hi, lo) split
                bp = 32 * (gi % 3)
                nc.tensor.matmul(ps[:, :W], neg_sb[bp:bp + 2, :],
                                 trsq_sb[bp:bp + 2, gi // 3, :W],
                                 start=True, stop=False)
                for k in range(KD):
                    nc.tensor.matmul(ps[:, :W], xT_sb[:, k, :], kT[:, k, :W],
                                     start=False, stop=(k == KD - 1))
                sc = sb.tile([B, 512], f32, tag=f"sc{gi}")
                sc_tiles.append(sc)
                WE = W if gi < 12 else W_LAST   # exclude pad cols from stats
                nc.scalar.copy(sc[:, :W], ps[:, :W])
                nc.vector.reduce_max(mpart[:, gi:gi + 1], sc[:, :WE],
                                     axis=mybir.AxisListType.X)

                # transposes lag one group so they don't gate the kT pipeline
                if pending is not None:
                    emit_transposes(*pending)
                pending = (kT, gi)
            if pending is not None:
                emit_transposes(*pending)

        # --- global max, bias = -g*M ---
        nc.vector.reduce_max(stat[:, 0:1], mpart[:, :NG],
                             axis=mybir.AxisListType.X)
        nc.vector.tensor_tensor(stat[:, 2:3], g_sb[:], stat[:, 0:1],
                                op=mybir.AluOpType.mult)
        nc.vector.tensor_scalar_mul(stat[:, 2:3], stat[:, 2:3], -1.0)

        # ---------------- phase 2: exp -> pT -> GEMM2 ----------------
        with tc.tile_pool(name="psT2", bufs=2, space="PSUM") as psT2, \
             tc.tile_pool(name="psA", bufs=1, space="PSUM") as psA:
            acc_ps = psA.tile([B, DJ, 512], f32)
            for c in range(KN):
                gi = c // 4
                ci = c % 4
                n0, W = GROUPS[gi]
                if ci == 0:
                    WE = W if gi < 12 else W_LAST
                    p = pp.tile([B, 512], f16, tag="p")
                    p_tiles.append(p)
                    if WE < W:
                        nc.vector.memset(p[:, WE:W], 0.0)
                    nc.scalar.activation(p[:, :WE], sc_tiles[gi][:, :WE],
                                         mybir.ActivationFunctionType.Exp,
                                         bias=stat[:, 2:3], scale=g_sb[:],
                                         accum_out=ssum[:, gi:gi + 1])
                pt2 = psT2.tile([128, B], f16, tag="pt2")
                nc.tensor.transpose(pt2[:],
                                    p_tiles[gi][:, ci * 128:(ci + 1) * 128],
                                    id_sb[:B, :B])
                pT = pTp.tile([128, B], f16, tag="pT")
                nc.vector.tensor_copy(pT[:], pt2[:])
                if c in nat_cache:
                    nat = nat_cache[c]
                else:
                    nat = natp.tile([128, D], f16, tag="nat")
                    nc.sync.dma_start(nat[:], tn[c * 128:(c + 1) * 128, :])
                for j in range(DJ):
                    nc.tensor.matmul(acc_ps[:, j, :], pT[:],
                                     nat[:, j * 512:(j + 1) * 512],
                                     start=(c == 0), stop=(c == KN - 1))
            for j in range(DJ):
                nc.scalar.copy(acc_sb[:, j * 512:(j + 1) * 512],
                               acc_ps[:, j, :])
                nc.sync.dma_start(acc_out[:, j * 512:(j + 1) * 512],
                                  acc_sb[:, j * 512:(j + 1) * 512])

        nc.vector.reduce_sum(stat[:, 1:2], ssum[:, :NG],
                             axis=mybir.AxisListType.X)
        nc.sync.dma_start(s_out[:], stat[:, 1:2])
        nc.sync.dma_start(m_out[:], stat[:, 0:1])

    nc.compile()
    return nc


def _get_nc():
    if "nc" not in _CACHED:
        _CACHED["nc"] = _build_nc()
    return _CACHED["nc"]


def kernel(x, train, alphas_cumprod, t, **_unused):
    from concourse.bass_utils import run_bass_kernel_spmd

    x = np.asarray(x)
    train = np.asarray(train)
    alphas_cumprod = np.asarray(alphas_cumprod)
    t = np.asarray(t).astype(np.int64)

    xf = x.reshape(B, -1).astype(np.float32)
    tf = train.reshape(N, -1).astype(np.float32)

    acp_t = alphas_cumprod.astype(np.float64)[t]
    a = np.sqrt(acp_t)
    om = 1.0 - acp_t
    gp32 = (a * a / (2.0 * om)).astype(np.float32)   # softmax scale on s''
    xscale = (2.0 / a).astype(np.float32)            # fold into x

    trsq_full = np.einsum("nd,nd->n", tf.astype(np.float64),
                          tf.astype(np.float64)).astype(np.float32)

    t16 = tf.astype(np.float16)
    x16T = np.ascontiguousarray(
        (xscale[:, None] * xf).astype(np.float16).T)  # [D, B]
    ident = np.eye(128, dtype=np.float16)
    g_col = gp32.reshape(B, 1)
    negs = np.zeros((96, B), dtype=np.float16)
    negs[[0, 1, 32, 33, 64, 65], :] = -1.0

    in_maps = []
    for c in range(NCORES):
        sl = slice(c * N_LOC, (c + 1) * N_LOC)
        nat = np.zeros((N_PAD, D), dtype=np.float16)
        nat[:N_LOC] = t16[sl]
        # group-contiguous transposed layout: per group g, [128 d-part, 24 k, W n]
        tTg = np.empty((12, 128, KD, 512), dtype=np.float16)
        for g in range(12):
            blk = nat[g * 512:(g + 1) * 512, :]          # [512 n, 3072 d]
            tTg[g] = blk.reshape(512, KD, 128).transpose(2, 1, 0)
        tTl = nat[6144:6272, :].reshape(128, KD, 128).transpose(2, 1, 0)
        trsq_c = np.zeros((N_PAD,), dtype=np.float32)
        trsq_c[:N_LOC] = trsq_full[sl]
        hi = trsq_c.astype(np.float16)
        lo = (trsq_c.astype(np.float64) - hi.astype(np.float64)).astype(np.float16)
        trsqg = np.zeros((96, 5, 512), dtype=np.float16)
        for g in range(NG):
            w = GROUPS[g][1]
            trsqg[32 * (g % 3), g // 3, :w] = hi[g * 512:g * 512 + w]
            trsqg[32 * (g % 3) + 1, g // 3, :w] = lo[g * 512:g * 512 + w]
        in_maps.append({
            "tTg": np.ascontiguousarray(tTg.reshape(12 * 128, KD * 512)),
            "tTl": np.ascontiguousarray(tTl.reshape(128, KD * 128)),
            "tn": nat,
            "xT": x16T,
            "ident": ident,
            "trsq": trsqg.reshape(96, 5 * 512),
            "negs": negs,
            "gcol": g_col,
        })

    nc = _get_nc()
    res = run_bass_kernel_spmd(nc, in_maps, list(range(NCORES)))
    _CACHED["last_results"] = res

    # --- host-side online-softmax merge across cores (fp64) ---
    g64 = gp32.astype(np.float64)
    Ms = np.stack([res.results[c]["m_out"][:, 0].astype(np.float64)
                   for c in range(NCORES)])          # [C, B]
    Ss = np.stack([res.results[c]["s_out"][:, 0].astype(np.float64)
                   for c in range(NCORES)])          # [C, B]
    ACCs = np.stack([res.results[c]["acc_out"].astype(np.float64)
                     for c in range(NCORES)])        # [C, B, D]
    Mg = Ms.max(axis=0)                              # [B]
    scale = np.exp(g64[None, :] * (Ms - Mg[None, :]))  # [C, B]
    den = (scale * Ss).sum(axis=0)                   # [B]
    num = (scale[:, :, None] * ACCs).sum(axis=0)     # [C, B, D] -> [B, D]
    weighted = num / den[:, None]

    coef_x = 1.0 / np.sqrt(om)
    coef_x_hat = a / np.sqrt(om)
    out = coef_x[:, None] * xf.astype(np.float64) - coef_x_hat[:, None] * weighted
    return out.reshape(x.shape).astype(np.float32)
